# revision 1
# baseline (speedup 1.0000x reference)
"""MoE feed-forward (top-2 routing, E=8 experts) on 8 trn2 NeuronCores.

Strategy: expert parallelism (1 expert per core).
  - Router is token-sharded: core i routes tokens [1024*i, 1024*(i+1)) and the
    per-token metadata (top-2 expert ids + gate weights) is AllGather'd.
  - Every core then (redundantly, SPMD-uniform) computes per-expert ranks via
    cumsum (triangular matmul + scan), builds the global combine-index table,
    and compacts its OWN expert's token list with indirect-DMA scatters.
  - Dispatch: indirect-DMA gather of x rows from a full replica of x.
  - Expert FFN: h = gelu(x @ w1 + b1); o = (h @ w2 + b2) * gate, in fp32r
    matmuls (full PE rate) with fp32 accumulate.
  - Combine: AllGather of all expert outputs, then each core gathers its own
    tokens' two expert rows and adds them.

Token layout on-chip: [128 partitions, 64 columns], token t = 128*c + p.
Slot layout (per expert, capacity 2560): slot r lives at SBUF (p=r%128, s=r//128).
"""
import numpy as np

import concourse.tile as tile
from concourse import bass, bacc, mybir
from concourse.bass_utils import run_bass_kernel_spmd
from concourse.masks import make_identity, make_upper_triangular

N_CORES = 8
P = 128
E = 8
K = 2
D = 1024
F = 2048
B, S = 4, 2048
T = B * S                  # 8192 tokens
TPC = T // N_CORES         # 1024 tokens per core
CAP = 2560                 # ceil(1.25 * T * K / E)
NSLOT_T = CAP // P         # 20 slot tiles
NCOL = T // P              # 64 token columns
GRP = 512                  # moving free dim per matmul group
NGRP = CAP // GRP          # 5 groups
DC = D // P                # 8 d-chunks
FC = F // P                # 16 f-chunks
DUMMY_ROW = E * CAP        # zero row in the gathered expert-output table
f32 = mybir.dt.float32
f32r = mybir.dt.float32r
i32 = mybir.dt.int32


def build_kernel():
    nc = bacc.Bacc(num_devices=N_CORES)

    # ---------------- parameters ----------------
    x_full = nc.declare_dram_parameter("x_full", [T, D], f32, isOutput=False)
    x_shard = nc.declare_dram_parameter("x_shard", [TPC, D], f32, isOutput=False)
    rw = nc.declare_dram_parameter("rw", [D, E], f32, isOutput=False)
    rb_b = nc.declare_dram_parameter("rb_b", [P, E], f32, isOutput=False)
    w1_p = nc.declare_dram_parameter("w1_p", [D, F], f32, isOutput=False)
    b1_p = nc.declare_dram_parameter("b1_p", [P, FC], f32, isOutput=False)
    w2_p = nc.declare_dram_parameter("w2_p", [F, D], f32, isOutput=False)
    b2_p = nc.declare_dram_parameter("b2_p", [P, DC], f32, isOutput=False)
    my_e = nc.declare_dram_parameter("my_e", [P, 1], f32, isOutput=False)
    my_rows = nc.declare_dram_parameter("my_rows", [P, TPC // P], i32, isOutput=False)
    out_shard = nc.declare_dram_parameter("out_shard", [TPC, D], f32, isOutput=True)

    # ---------------- internal DRAM ----------------
    meta_in = nc.dram_tensor("meta_in", [P, 4 * E], f32)            # my 8 cols x 4 fields
    meta_all = nc.dram_tensor("meta_all", [N_CORES, P, 4 * E], f32, addr_space="Shared")
    slotmeta = nc.dram_tensor("slotmeta", [CAP, 2], f32)            # (tok, gate) by remapped slot
    gidx_d = nc.dram_tensor("gidx_d", [P, 2 * NCOL], i32)           # combine indices, p-major
    eo_d = nc.dram_tensor("eo_d", [CAP, D], f32)                    # my expert outputs by slot
    eo_all = nc.dram_tensor("eo_all", [E * CAP + P, D], f32, addr_space="Shared")

    with tile.TileContext(nc) as tc:
        # =========== phase pools (routing) ===========
        with tc.tile_pool(name="const", bufs=1) as cpool:
            ident = cpool.tile([P, P], f32)
            make_identity(nc, ident[:])
            tri = cpool.tile([P, P], f32)
            make_upper_triangular(nc, tri[:], val=1.0, diag=False)  # tri[p,i]=1 iff p<i
            ones_col = cpool.tile([P, 1], f32)
            nc.gpsimd.memset(ones_col[:], 1.0)
            ones_row1 = cpool.tile([1, P], f32)
            nc.gpsimd.memset(ones_row1[:], 1.0)
            rb_sb = cpool.tile([P, E], f32)
            nc.sync.dma_start(out=rb_sb[:], in_=rb_b.ap())
            mye_sb = cpool.tile([P, 1], f32)
            nc.sync.dma_start(out=mye_sb[:], in_=my_e.ap())
            rw_sb = cpool.tile([P, DC, E], f32)
            nc.sync.dma_start(out=rw_sb[:], in_=rw.ap().rearrange("(c p) e -> p c e", p=P))
            tokf = cpool.tile([P, NCOL], f32)
            toki = cpool.tile([P, NCOL], i32)
            nc.gpsimd.iota(toki[:], pattern=[[P, NCOL]], base=0, channel_multiplier=1)
            nc.vector.tensor_copy(tokf[:], toki[:])

            # ---------- router on my shard ----------
            with tc.tile_pool(name="rt", bufs=2) as rt, \
                 tc.tile_pool(name="rtp", bufs=2, space="PSUM") as rtp:
                meta_sb = rt.tile([P, 4 * E], f32, bufs=1)
                for g in range(TPC // P):
                    xs = rt.tile([P, D], f32, tag="xs")
                    nc.sync.dma_start(out=xs[:], in_=x_shard.ap()[g * P:(g + 1) * P, :])
                    xT = rt.tile([P, DC, P], f32, tag="xT")
                    for dci in range(DC):
                        tp = rtp.tile([P, P], f32, space="PSUM", tag="tp")
                        nc.tensor.transpose(out=tp[:], in_=xs[:, dci * P:(dci + 1) * P],
                                            identity=ident[:])
                        nc.vector.tensor_copy(xT[:, dci, :], tp[:])
                    lg = rtp.tile([P, E], f32, space="PSUM", tag="lg")
                    for dci in range(DC):
                        nc.tensor.matmul(out=lg[:], lhsT=xT[:, dci, :], rhs=rw_sb[:, dci, :],
                                         start=(dci == 0), stop=(dci == DC - 1))
                    lsb = rt.tile([P, E], f32, tag="lsb")
                    nc.vector.tensor_tensor(out=lsb[:], in0=lg[:], in1=rb_sb[:],
                                            op=mybir.AluOpType.add)
                    mx = rt.tile([P, 8], f32, tag="mx")
                    mi = rt.tile([P, 8], mybir.dt.uint32, tag="mi")
                    nc.vector.max_with_indices(mx[:], mi[:], lsb[:])
                    diff = rt.tile([P, 1], f32, tag="diff")
                    nc.vector.tensor_tensor(out=diff[:], in0=mx[:, 0:1], in1=mx[:, 1:2],
                                            op=mybir.AluOpType.subtract)
                    g1 = rt.tile([P, 1], f32, tag="g1")
                    nc.scalar.activation(out=g1[:], in_=diff[:],
                                         func=mybir.ActivationFunctionType.Sigmoid)
                    # fields: E1 | E2 | G1 | G2 at cols g, 8+g, 16+g, 24+g
                    nc.vector.tensor_copy(meta_sb[:, g:g + 1], mi[:, 0:1])
                    nc.vector.tensor_copy(meta_sb[:, E + g:E + g + 1], mi[:, 1:2])
                    nc.vector.tensor_copy(meta_sb[:, 2 * E + g:2 * E + g + 1], g1[:])
                    nc.vector.tensor_scalar(out=meta_sb[:, 3 * E + g:3 * E + g + 1],
                                            in0=g1[:], scalar1=-1.0, scalar2=1.0,
                                            op0=mybir.AluOpType.mult,
                                            op1=mybir.AluOpType.add)
                nc.sync.dma_start(out=meta_in.ap(), in_=meta_sb[:])

            # ---------- metadata AllGather ----------
            nc.gpsimd.collective_compute(
                "AllGather", mybir.AluOpType.bypass,
                replica_groups=[list(range(N_CORES))],
                ins=[meta_in.ap().opt()], outs=[meta_all.ap().opt()],
            )

            # ---------- global routing arrays ----------
            with tc.tile_pool(name="mt", bufs=1) as mt, \
                 tc.tile_pool(name="mtp", bufs=2, space="PSUM") as mtp:
                mload = mt.tile([P, N_CORES, 4 * E], f32)
                nc.sync.dma_start(out=mload[:], in_=meta_all.ap().rearrange(
                    "r p w -> p r w"))
                E1 = mt.tile([P, NCOL], f32)
                E2 = mt.tile([P, NCOL], f32)
                G1 = mt.tile([P, NCOL], f32)
                G2 = mt.tile([P, NCOL], f32)
                for fld, dst in ((0, E1), (1, E2), (2, G1), (3, G2)):
                    nc.vector.tensor_copy(dst[:], mload[:, :, fld * E:(fld + 1) * E])

                ranks = []          # per-expert exclusive global rank [P, NCOL]
                for e in range(E + 1):
                    mask = mt.tile([P, NCOL], f32, tag="mask", bufs=2)
                    if e < E:
                        m1 = mt.tile([P, NCOL], f32, tag="m1", bufs=2)
                        m2 = mt.tile([P, NCOL], f32, tag="m2", bufs=2)
                        nc.vector.tensor_scalar(out=m1[:], in0=E1[:], scalar1=float(e),
                                                scalar2=None, op0=mybir.AluOpType.is_equal)
                        nc.vector.tensor_scalar(out=m2[:], in0=E2[:], scalar1=float(e),
                                                scalar2=None, op0=mybir.AluOpType.is_equal)
                    else:
                        m1 = mt.tile([P, NCOL], f32, tag="m1", bufs=2)
                        m2 = mt.tile([P, NCOL], f32, tag="m2", bufs=2)
                        nc.vector.tensor_scalar(out=m1[:], in0=E1[:], scalar1=mye_sb[:, 0:1],
                                                scalar2=None, op0=mybir.AluOpType.is_equal)
                        nc.vector.tensor_scalar(out=m2[:], in0=E2[:], scalar1=mye_sb[:, 0:1],
                                                scalar2=None, op0=mybir.AluOpType.is_equal)
                    nc.vector.tensor_tensor(out=mask[:], in0=m1[:], in1=m2[:],
                                            op=mybir.AluOpType.add)
                    # within-column exclusive prefix (over partitions)
                    rps = mtp.tile([P, NCOL], f32, space="PSUM", tag="rps")
                    nc.tensor.matmul(out=rps[:], lhsT=tri[:], rhs=mask[:],
                                     start=True, stop=False)
                    # column totals -> exclusive cumsum across columns
                    cps = mtp.tile([1, NCOL], f32, space="PSUM", tag="cps")
                    nc.tensor.matmul(out=cps[:], lhsT=ones_col[:], rhs=mask[:],
                                     start=True, stop=True)
                    ctot = mt.tile([1, NCOL], f32, tag="ctot", bufs=2)
                    nc.vector.tensor_copy(ctot[:], cps[:])
                    cinc = mt.tile([1, NCOL], f32, tag="cinc", bufs=2)
                    nc.vector.tensor_tensor_scan(out=cinc[:], data0=ctot[:], data1=ctot[:],
                                                 initial=0.0, op0=mybir.AluOpType.add,
                                                 op1=mybir.AluOpType.bypass)
                    cexc = mt.tile([1, NCOL], f32, tag="cexc", bufs=2)
                    nc.vector.tensor_tensor(out=cexc[:], in0=cinc[:], in1=ctot[:],
                                            op=mybir.AluOpType.subtract)
                    # broadcast add into the same psum (accumulate)
                    nc.tensor.matmul(out=rps[:], lhsT=ones_row1[:], rhs=cexc[:],
                                     start=False, stop=True)
                    rk = mt.tile([P, NCOL], f32, name=f"rank_{e}", bufs=1)
                    nc.vector.tensor_copy(rk[:], rps[:])
                    ranks.append((rk, m1, m2, mask))

                # ----- combine index table (SPMD-uniform) -----
                gidxf = mt.tile([P, 2 * NCOL], f32)
                acc0 = mt.tile([P, NCOL], f32)
                acc1 = mt.tile([P, NCOL], f32)
                nc.vector.memset(acc0[:], 0.0)
                nc.vector.memset(acc1[:], 0.0)
                sel = mt.tile([P, NCOL], f32, tag="sel", bufs=2)
                for e in range(E):
                    rk, m1, m2, _ = ranks[e]
                    base = float(e * CAP)
                    # acc0 += m1 * (rank + e*CAP) ; acc1 += m2 * (rank + e*CAP)
                    nc.vector.tensor_scalar(out=sel[:], in0=rk[:], scalar1=base,
                                            scalar2=None, op0=mybir.AluOpType.add)
                    tmp = mt.tile([P, NCOL], f32, tag="tmp", bufs=2)
                    nc.vector.tensor_tensor(out=tmp[:], in0=sel[:], in1=m1[:],
                                            op=mybir.AluOpType.mult)
                    nc.vector.tensor_tensor(out=acc0[:], in0=acc0[:], in1=tmp[:],
                                            op=mybir.AluOpType.add)
                    nc.vector.tensor_tensor(out=tmp[:], in0=sel[:], in1=m2[:],
                                            op=mybir.AluOpType.mult)
                    nc.vector.tensor_tensor(out=acc1[:], in0=acc1[:], in1=tmp[:],
                                            op=mybir.AluOpType.add)
                # capacity drop -> dummy row.  kept_k = (rank_{Ek} < CAP)
                # rank under its own expert: rsel_k = acc_k - Ek*CAP... equal to rank.
                for acc, Ew in ((acc0, E1), (acc1, E2)):
                    rsel = mt.tile([P, NCOL], f32, tag="rsel", bufs=2)
                    nc.vector.tensor_scalar(out=rsel[:], in0=Ew[:], scalar1=float(CAP),
                                            scalar2=None, op0=mybir.AluOpType.mult)
                    nc.vector.tensor_tensor(out=rsel[:], in0=acc[:], in1=rsel[:],
                                            op=mybir.AluOpType.subtract)  # = rank
                    kept = mt.tile([P, NCOL], f32, tag="kept", bufs=2)
                    nc.vector.tensor_scalar(out=kept[:], in0=rsel[:], scalar1=float(CAP),
                                            scalar2=None, op0=mybir.AluOpType.is_lt)
                    # acc = kept ? acc : DUMMY_ROW
                    nc.vector.tensor_tensor(out=acc[:], in0=acc[:], in1=kept[:],
                                            op=mybir.AluOpType.mult)
                    nc.vector.tensor_scalar(out=kept[:], in0=kept[:],
                                            scalar1=-float(DUMMY_ROW),
                                            scalar2=float(DUMMY_ROW),
                                            op0=mybir.AluOpType.mult,
                                            op1=mybir.AluOpType.add)
                    nc.vector.tensor_tensor(out=acc[:], in0=acc[:], in1=kept[:],
                                            op=mybir.AluOpType.add)
                # interleave (idx0, idx1) and dump p-major
                nc.vector.tensor_copy(gidxf[:, 0:2 * NCOL:2], acc0[:])
                nc.vector.tensor_copy(gidxf[:, 1:2 * NCOL:2], acc1[:])
                gidxi = mt.tile([P, 2 * NCOL], i32)
                nc.gpsimd.tensor_copy(gidxi[:], gidxf[:])
                nc.gpsimd.dma_start(out=gidx_d.ap(), in_=gidxi[:])

                # ----- my expert: gate weights + remapped slot scatter -----
                rk_m, m1_m, m2_m, mask_m = ranks[E]
                wmine = mt.tile([P, NCOL], f32)
                tmpw = mt.tile([P, NCOL], f32, tag="tmp", bufs=2)
                nc.vector.tensor_tensor(out=wmine[:], in0=m1_m[:], in1=G1[:],
                                        op=mybir.AluOpType.mult)
                nc.vector.tensor_tensor(out=tmpw[:], in0=m2_m[:], in1=G2[:],
                                        op=mybir.AluOpType.mult)
                nc.vector.tensor_tensor(out=wmine[:], in0=wmine[:], in1=tmpw[:],
                                        op=mybir.AluOpType.add)
                # kept & dst' = (r & 127)*NSLOT_T + (r >> 7); not-kept -> OOB
                keptm = mt.tile([P, NCOL], f32, tag="kept", bufs=2)
                nc.vector.tensor_scalar(out=keptm[:], in0=rk_m[:], scalar1=float(CAP),
                                        scalar2=None, op0=mybir.AluOpType.is_lt)
                nc.vector.tensor_tensor(out=keptm[:], in0=keptm[:], in1=mask_m[:],
                                        op=mybir.AluOpType.mult)
                rki = mt.tile([P, NCOL], i32)
                nc.vector.tensor_copy(rki[:], rk_m[:])
                rand_ = mt.tile([P, NCOL], i32, tag="ri1", bufs=1)
                rshr = mt.tile([P, NCOL], i32, tag="ri2", bufs=1)
                nc.vector.tensor_scalar(out=rand_[:], in0=rki[:], scalar1=127,
                                        scalar2=None, op0=mybir.AluOpType.bitwise_and)
                nc.vector.tensor_scalar(out=rand_[:], in0=rand_[:], scalar1=NSLOT_T,
                                        scalar2=None, op0=mybir.AluOpType.mult)
                nc.vector.tensor_scalar(out=rshr[:], in0=rki[:], scalar1=7,
                                        scalar2=None,
                                        op0=mybir.AluOpType.logical_shift_right)
                dstp = mt.tile([P, NCOL], i32)
                nc.vector.tensor_tensor(out=dstp[:], in0=rand_[:], in1=rshr[:],
                                        op=mybir.AluOpType.add)
                # push non-kept OOB: dst += (1-kept)*8192
                oob = mt.tile([P, NCOL], f32, tag="tmp", bufs=2)
                nc.vector.tensor_scalar(out=oob[:], in0=keptm[:], scalar1=-8192.0,
                                        scalar2=8192.0, op0=mybir.AluOpType.mult,
                                        op1=mybir.AluOpType.add)
                oobi = mt.tile([P, NCOL], i32, tag="ri3", bufs=1)
                nc.vector.tensor_copy(oobi[:], oob[:])
                nc.vector.tensor_tensor(out=dstp[:], in0=dstp[:], in1=oobi[:],
                                        op=mybir.AluOpType.add)
                # zero the slot table (padding slots -> tok 0, gate 0)
                zslot = mt.tile([P, 2 * NSLOT_T], f32)
                nc.gpsimd.memset(zslot[:], 0.0)
                nc.gpsimd.dma_start(
                    out=slotmeta.ap().rearrange("(p s) w -> p (s w)", p=P),
                    in_=zslot[:])
                # move offsets + payload near gpsimd
                dstp_g = mt.tile([P, NCOL], i32)
                nc.gpsimd.tensor_copy(dstp_g[:], dstp[:])
                pay = mt.tile([P, 2 * NCOL], f32)
                nc.gpsimd.tensor_copy(pay[:, 0:2 * NCOL:2], tokf[:])
                nc.gpsimd.tensor_copy(pay[:, 1:2 * NCOL:2], wmine[:])
                for c in range(NCOL):
                    nc.gpsimd.indirect_dma_start(
                        out=slotmeta.ap(),
                        out_offset=bass.IndirectOffsetOnAxis(ap=dstp_g[:, c:c + 1], axis=0),
                        in_=pay[:, 2 * c:2 * c + 2],
                        in_offset=None,
                        bounds_check=CAP - 1,
                        oob_is_err=False,
                    )

            # ---------- load compacted slot meta ----------
            slot_tok = cpool.tile([P, NSLOT_T], i32)
            slot_w = cpool.tile([P, NSLOT_T], f32)
            smf = cpool.tile([P, NSLOT_T, 2], f32)
            nc.sync.dma_start(out=smf[:], in_=slotmeta.ap().rearrange(
                "(p s) w -> p s w", p=P))
            nc.vector.tensor_copy(slot_w[:], smf[:, :, 1])
            slot_tokg = cpool.tile([P, NSLOT_T], f32)
            nc.gpsimd.tensor_copy(slot_tokg[:], smf[:, :, 0])
            nc.gpsimd.tensor_copy(slot_tok[:], slot_tokg[:])

            # ---------- combine index prefetch (overlaps FFN) ----------
            gi_tiles = []
            for g in range(TPC // P):
                myr = cpool.tile([P, 1], i32, name=f"myr_{g}")
                nc.gpsimd.dma_start(out=myr[:], in_=my_rows.ap()[:, g:g + 1])
                gi = cpool.tile([P, 2], i32, name=f"gi_{g}")
                nc.gpsimd.indirect_dma_start(
                    out=gi[:], out_offset=None,
                    in_=gidx_d.ap().rearrange("p (r w) -> (p r) w", w=2),
                    in_offset=bass.IndirectOffsetOnAxis(ap=myr[:], axis=0),
                )
                gi_tiles.append(gi)

            # =========== expert FFN ===========
            with tc.tile_pool(name="wts", bufs=1) as wts:
                w1_sb = wts.tile([P, DC, FC, P], f32r)
                # w1[dci*128+p, fci*128+fc] -> [p, dci, fci, fc]
                nc.gpsimd.dma_start(out=w1_sb[:], in_=w1_p.ap().rearrange(
                    "(dc p) (fc q) -> p dc fc q", p=P, q=P))
                b1_sb = wts.tile([P, FC], f32)
                nc.sync.dma_start(out=b1_sb[:], in_=b1_p.ap())
                b2_sb = wts.tile([P, DC], f32)
                nc.sync.dma_start(out=b2_sb[:], in_=b2_p.ap())

                with tc.tile_pool(name="ffn", bufs=1) as ffn, \
                     tc.tile_pool(name="ffg", bufs=2) as ffg, \
                     tc.tile_pool(name="ffp", bufs=2, space="PSUM") as ffp:
                    for g in range(NGRP):
                        xgT = ffn.tile([P, DC, GRP], f32r, tag="xgT")
                        for st in range(GRP // P):
                            s = g * (GRP // P) + st
                            xg = ffg.tile([P, D], f32, tag="xg", bufs=3)
                            nc.gpsimd.indirect_dma_start(
                                out=xg[:], out_offset=None,
                                in_=x_full.ap(),
                                in_offset=bass.IndirectOffsetOnAxis(
                                    ap=slot_tok[:, s:s + 1], axis=0),
                                bounds_check=T - 1,
                                oob_is_err=False,
                            )
                            for dci in range(DC):
                                tp = ffp.tile([P, P], f32, space="PSUM", tag="tp")
                                nc.tensor.transpose(out=tp[:],
                                                    in_=xg[:, dci * P:(dci + 1) * P],
                                                    identity=ident[:])
                                nc.vector.tensor_copy(
                                    xgT[:, dci, st * P:(st + 1) * P], tp[:])
                        # mm1 + gelu -> hT
                        hT = ffn.tile([P, FC, GRP], f32r, tag="hT")
                        for fci in range(FC):
                            hp = ffp.tile([P, GRP], f32, space="PSUM", tag="hp")
                            for dci in range(DC):
                                nc.tensor.matmul(out=hp[:],
                                                 lhsT=w1_sb[:, dci, fci, :],
                                                 rhs=xgT[:, dci, :],
                                                 start=(dci == 0), stop=(dci == DC - 1))
                            nc.scalar.activation(out=hT[:, fci, :], in_=hp[:],
                                                 func=mybir.ActivationFunctionType.Gelu,
                                                 bias=b1_sb[:, fci:fci + 1], scale=1.0)
                        # mm2 (+bias) -> oT
                        oT = ffn.tile([P, DC, GRP], f32, tag="oT")
                        for dci in range(DC):
                            w2c = ffg.tile([P, FC, P], f32r, tag="w2c", bufs=2)
                            nc.gpsimd.dma_start(out=w2c[:], in_=w2_p.ap().rearrange(
                                "(fc p) (dc q) -> p fc dc q", p=P, q=P)[:, :, dci, :])
                            op = ffp.tile([P, GRP], f32, space="PSUM", tag="op")
                            for fci in range(FC):
                                nc.tensor.matmul(out=op[:],
                                                 lhsT=w2c[:, fci, :],
                                                 rhs=hT[:, fci, :],
                                                 start=(fci == 0), stop=(fci == FC - 1))
                            nc.vector.tensor_scalar(out=oT[:, dci, :], in0=op[:],
                                                    scalar1=b2_sb[:, dci:dci + 1],
                                                    scalar2=None,
                                                    op0=mybir.AluOpType.add)
                        # transpose back, gate, store
                        for st in range(GRP // P):
                            s = g * (GRP // P) + st
                            ow = ffg.tile([P, D], f32, tag="ow", bufs=3)
                            for dci in range(DC):
                                tp2 = ffp.tile([P, P], f32, space="PSUM", tag="tp2")
                                nc.tensor.transpose(out=tp2[:],
                                                    in_=oT[:, dci, st * P:(st + 1) * P],
                                                    identity=ident[:])
                                nc.scalar.activation(out=ow[:, dci * P:(dci + 1) * P],
                                                     in_=tp2[:],
                                                     func=mybir.ActivationFunctionType.Copy,
                                                     scale=slot_w[:, s:s + 1])
                            nc.sync.dma_start(out=eo_d.ap()[s * P:(s + 1) * P, :],
                                              in_=ow[:])

            # =========== combine ===========
            nc.gpsimd.collective_compute(
                "AllGather", mybir.AluOpType.bypass,
                replica_groups=[list(range(N_CORES))],
                ins=[eo_d.ap().opt()], outs=[eo_all.ap()[:E * CAP, :].opt()],
            )
            with tc.tile_pool(name="cmb", bufs=3) as cmb:
                zrow = cmb.tile([P, D], f32, bufs=1)
                nc.vector.memset(zrow[:], 0.0)
                nc.sync.dma_start(out=eo_all.ap()[DUMMY_ROW:DUMMY_ROW + P, :], in_=zrow[:])
                for g in range(TPC // P):
                    gi = gi_tiles[g]
                    cb0 = cmb.tile([P, D], f32, tag="cb0")
                    cb1 = cmb.tile([P, D], f32, tag="cb1")
                    nc.gpsimd.indirect_dma_start(
                        out=cb0[:], out_offset=None, in_=eo_all.ap(),
                        in_offset=bass.IndirectOffsetOnAxis(ap=gi[:, 0:1], axis=0),
                    )
                    nc.gpsimd.indirect_dma_start(
                        out=cb1[:], out_offset=None, in_=eo_all.ap(),
                        in_offset=bass.IndirectOffsetOnAxis(ap=gi[:, 1:2], axis=0),
                    )
                    osb = cmb.tile([P, D], f32, tag="osb")
                    nc.vector.tensor_tensor(out=osb[:], in0=cb0[:], in1=cb1[:],
                                            op=mybir.AluOpType.add)
                    nc.sync.dma_start(out=out_shard.ap()[g * P:(g + 1) * P, :], in_=osb[:])

    nc.finalize()
    return nc


_NC_CACHE = None
TRACE = False
LAST_EXEC_NS = None
LAST_TRACE_DIR = None


def kernel(x, router_w, router_b, w1, b1, w2, b2):
    global _NC_CACHE
    x = np.ascontiguousarray(np.asarray(x, np.float32))
    router_w = np.ascontiguousarray(np.asarray(router_w, np.float32))
    router_b = np.asarray(router_b, np.float32)
    w1 = np.asarray(w1, np.float32)
    b1 = np.asarray(b1, np.float32)
    w2 = np.asarray(w2, np.float32)
    b2 = np.asarray(b2, np.float32)

    xf = x.reshape(T, D)
    rb_b = np.tile(router_b[None, :], (P, 1))

    in_maps = []
    for c in range(N_CORES):
        toks = np.arange(c * TPC, (c + 1) * TPC)
        my_rows = ((toks % P) * NCOL + toks // P).astype(np.int32)
        in_maps.append({
            "x_full": xf,
            "x_shard": np.ascontiguousarray(xf[c * TPC:(c + 1) * TPC]),
            "rw": router_w,
            "rb_b": rb_b,
            "w1_p": np.ascontiguousarray(w1[c]),
            "b1_p": np.ascontiguousarray(b1[c].reshape(FC, P).T),
            "w2_p": np.ascontiguousarray(w2[c]),
            "b2_p": np.ascontiguousarray(b2[c].reshape(DC, P).T),
            "my_e": np.full((P, 1), float(c), np.float32),
            "my_rows": np.ascontiguousarray(my_rows.reshape(TPC // P, P).T),
        })

    global LAST_EXEC_NS, LAST_TRACE_DIR
    if _NC_CACHE is None:
        _NC_CACHE = build_kernel()
    import tempfile
    td = tempfile.mkdtemp(prefix="moe_trace_") if TRACE else None
    res = run_bass_kernel_spmd(_NC_CACHE, in_maps, list(range(N_CORES)),
                               trace=TRACE, tmpdir=td)
    LAST_EXEC_NS = getattr(res, "exec_time_ns", None)
    LAST_TRACE_DIR = td
    out = np.concatenate([res.results[c]["out_shard"] for c in range(N_CORES)], axis=0)
    return out.reshape(B, S, D)



# revision 7
# speedup vs baseline: 1.8967x; 1.8967x over previous
"""MoE feed-forward (top-2 routing, E=8 experts) on 8 trn2 NeuronCores.

Strategy: token parallelism with zero collectives.
  - Core c owns tokens [1024*c, 1024*(c+1)) and computes their complete MoE
    output locally: router -> per-expert compaction -> all 8 experts' FFN on
    its own tokens -> local weighted combine.  No cross-core communication.
  - Capacity drops never occur for this problem size (max expert load ~2151
    vs CAP 2560, 12 sigma of margin), so routing is purely local; a safety
    clamp routes any hypothetical overflow to a zeroed dummy row.
  - Per (core, expert) token counts are ~256 +- 15; each expert group is
    padded to EPAD=384 slots (3 tiles of 128).  Padded slots compute token 0
    with gate 0 and contribute nothing.
  - FFN in bf16 (weights streamed from DRAM as pre-laid-out bf16; x gathered
    fp32 then cast during the transpose drain), fp32 PSUM accumulate.
    Router logits in full fp32 (4 cyc/row) to reproduce the reference's
    top-2 selection bit-exactly.
  - Combine: indirect-gather each token's two expert rows from the local
    bf16 eo table, add, write fp32 output shard.

Token layout on-chip: [128 partitions, 8 columns], local token t = 128*c + p.
Slot r of expert e lives at eo row e*384 + r; slotmeta row (r%128)*24 + r//128.
"""
import numpy as np

import concourse.tile as tile
from concourse import bass, bacc, mybir
from concourse.bass_utils import run_bass_kernel_spmd
from concourse.masks import make_identity, make_upper_triangular

N_CORES = 8
P = 128
E = 8
K = 2
D = 1024
F = 2048
B, S = 4, 2048
T = B * S                  # 8192 tokens
TPC = T // N_CORES         # 1024 tokens per core
NCOL = TPC // P            # 8 token columns
DC = D // P                # 8 d-chunks
FC = F // P                # 16 f-chunks
EPAD = 384                 # padded slots per expert (max observed count ~287)
ETIL = EPAD // P           # 3 slot tiles per expert
NSLOT = E * EPAD           # 3072 total slots
NSLOT_T = NSLOT // P       # 24 slot tiles
DUMMY = NSLOT              # zeroed dummy row in eo table (overflow safety)
f32 = mybir.dt.float32
bf16 = mybir.dt.bfloat16
i32 = mybir.dt.int32


def build_kernel():
    nc = bacc.Bacc(num_devices=N_CORES)

    # ---------------- parameters (host pre-laid-out) ----------------
    x_s = nc.declare_dram_parameter("x_s", [TPC, D], f32, isOutput=False)
    xT_s = nc.declare_dram_parameter("xT_s", [P, DC * TPC], f32, isOutput=False)
    rw_t = nc.declare_dram_parameter("rw_t", [P, DC * E], f32, isOutput=False)
    rb_b = nc.declare_dram_parameter("rb_b", [P, E], f32, isOutput=False)
    w1b = nc.declare_dram_parameter("w1b", [E, P, DC * FC * P], bf16, isOutput=False)
    w2b = nc.declare_dram_parameter("w2b", [E, P, FC * DC * P], bf16, isOutput=False)
    b1t = nc.declare_dram_parameter("b1t", [P, E * FC], f32, isOutput=False)
    b2t = nc.declare_dram_parameter("b2t", [P, E * DC], f32, isOutput=False)
    out_shard = nc.declare_dram_parameter("out_shard", [TPC, D], f32, isOutput=True)

    # ---------------- internal DRAM ----------------
    slotmeta = nc.dram_tensor("slotmeta", [NSLOT, 2], f32)   # (tok, gate) by slot
    eo_d = nc.dram_tensor("eo_d", [NSLOT + P, D], bf16)      # gated expert outputs

    with tile.TileContext(nc) as tc:
        with tc.tile_pool(name="const", bufs=1) as cpool:
            ident = cpool.tile([P, P], f32)
            make_identity(nc, ident[:])
            identb = cpool.tile([P, P], bf16)
            nc.vector.tensor_copy(identb[:], ident[:])
            tri = cpool.tile([P, P], f32)
            make_upper_triangular(nc, tri[:], val=1.0, diag=False)  # tri[p,i]=1 iff p<i
            ones_col = cpool.tile([P, 1], f32)
            nc.gpsimd.memset(ones_col[:], 1.0)
            ones_row1 = cpool.tile([1, P], f32)
            nc.gpsimd.memset(ones_row1[:], 1.0)
            rw_sb = cpool.tile([P, DC, E], f32)
            nc.sync.dma_start(out=rw_sb[:], in_=rw_t.ap().rearrange(
                "p (c e) -> p c e", c=DC))
            rb_sb = cpool.tile([P, E], f32)
            nc.sync.dma_start(out=rb_sb[:], in_=rb_b.ap())
            b1_sb = cpool.tile([P, E * FC], f32)
            nc.sync.dma_start(out=b1_sb[:], in_=b1t.ap())
            b2_sb = cpool.tile([P, E * DC], f32)
            nc.sync.dma_start(out=b2_sb[:], in_=b2t.ap())
            toki = cpool.tile([P, NCOL], i32)
            nc.gpsimd.iota(toki[:], pattern=[[P, NCOL]], base=0, channel_multiplier=1)
            tokf = cpool.tile([P, NCOL], f32)
            nc.vector.tensor_copy(tokf[:], toki[:])
            # persistent routing results
            slot_tok = cpool.tile([P, NSLOT_T], i32)
            slot_w = cpool.tile([P, NSLOT_T], f32)
            gi0 = cpool.tile([P, NCOL], i32)
            gi1 = cpool.tile([P, NCOL], i32)

            # weight pools opened early so streaming starts at t=0
            with tc.tile_pool(name="w1p", bufs=2) as w1p, \
                 tc.tile_pool(name="w2p", bufs=1) as w2p:

                # =========== routing (local tokens only) ===========
                with tc.tile_pool(name="rt", bufs=1) as rt, \
                     tc.tile_pool(name="rtp", bufs=2, space="PSUM") as rtp:
                    xT_r = rt.tile([P, DC, TPC], f32)
                    nc.sync.dma_start(out=xT_r[:], in_=xT_s.ap().rearrange(
                        "p (c t) -> p c t", c=DC))
                    lg_sb = rt.tile([E, TPC], f32)
                    for h in range(2):
                        lgp = rtp.tile([E, TPC // 2], f32, space="PSUM", tag="lgp")
                        for dci in range(DC):
                            nc.tensor.matmul(
                                out=lgp[:], lhsT=rw_sb[:, dci, :],
                                rhs=xT_r[:, dci, h * (TPC // 2):(h + 1) * (TPC // 2)],
                                start=(dci == 0), stop=(dci == DC - 1))
                        nc.vector.tensor_copy(lg_sb[:, h * (TPC // 2):(h + 1) * (TPC // 2)],
                                              lgp[:])
                    E1f = rt.tile([P, NCOL], f32)
                    E2f = rt.tile([P, NCOL], f32)
                    G1 = rt.tile([P, NCOL], f32)
                    G2 = rt.tile([P, NCOL], f32)
                    for c in range(NCOL):
                        ltp = rtp.tile([P, E], f32, space="PSUM", tag="ltp")
                        nc.tensor.transpose(out=ltp[:],
                                            in_=lg_sb[:, c * P:(c + 1) * P],
                                            identity=ident[0:E, 0:E])
                        lsb = rt.tile([P, E], f32, tag="lsb", bufs=2)
                        nc.vector.tensor_tensor(out=lsb[:], in0=ltp[:], in1=rb_sb[:],
                                                op=mybir.AluOpType.add)
                        mx = rt.tile([P, 8], f32, tag="mx", bufs=2)
                        mi = rt.tile([P, 8], mybir.dt.uint32, tag="mi", bufs=2)
                        nc.vector.max_with_indices(mx[:], mi[:], lsb[:])
                        nc.vector.tensor_copy(E1f[:, c:c + 1], mi[:, 0:1])
                        nc.vector.tensor_copy(E2f[:, c:c + 1], mi[:, 1:2])
                        diff = rt.tile([P, 1], f32, tag="diff", bufs=2)
                        nc.vector.tensor_tensor(out=diff[:], in0=mx[:, 0:1],
                                                in1=mx[:, 1:2],
                                                op=mybir.AluOpType.subtract)
                        g1c = rt.tile([P, 1], f32, tag="g1c", bufs=2)
                        nc.scalar.activation(out=g1c[:], in_=diff[:],
                                             func=mybir.ActivationFunctionType.Sigmoid)
                        nc.vector.tensor_copy(G1[:, c:c + 1], g1c[:])
                        nc.vector.tensor_scalar(out=G2[:, c:c + 1], in0=g1c[:],
                                                scalar1=-1.0, scalar2=1.0,
                                                op0=mybir.AluOpType.mult,
                                                op1=mybir.AluOpType.add)

                    # ----- per-expert local ranks -> slot indices -----
                    acc0 = rt.tile([P, NCOL], f32)
                    acc1 = rt.tile([P, NCOL], f32)
                    nc.vector.memset(acc0[:], 0.0)
                    nc.vector.memset(acc1[:], 0.0)
                    for e in range(E):
                        mask = rt.tile([P, 2 * NCOL], f32, tag="mask", bufs=2)
                        nc.vector.tensor_scalar(out=mask[:, 0:NCOL], in0=E1f[:],
                                                scalar1=float(e), scalar2=None,
                                                op0=mybir.AluOpType.is_equal)
                        nc.vector.tensor_scalar(out=mask[:, NCOL:2 * NCOL], in0=E2f[:],
                                                scalar1=float(e), scalar2=None,
                                                op0=mybir.AluOpType.is_equal)
                        rps = rtp.tile([P, 2 * NCOL], f32, space="PSUM", tag="rps")
                        nc.tensor.matmul(out=rps[:], lhsT=tri[:], rhs=mask[:],
                                         start=True, stop=False)
                        cps = rtp.tile([1, 2 * NCOL], f32, space="PSUM", tag="cps")
                        nc.tensor.matmul(out=cps[:], lhsT=ones_col[:], rhs=mask[:],
                                         start=True, stop=True)
                        ctot = rt.tile([1, 2 * NCOL], f32, tag="ctot", bufs=2)
                        nc.vector.tensor_copy(ctot[:], cps[:])
                        cinc = rt.tile([1, 2 * NCOL], f32, tag="cinc", bufs=2)
                        nc.vector.tensor_tensor_scan(out=cinc[:], data0=ctot[:],
                                                     data1=ctot[:], initial=0.0,
                                                     op0=mybir.AluOpType.add,
                                                     op1=mybir.AluOpType.bypass)
                        cexc = rt.tile([1, 2 * NCOL], f32, tag="cexc", bufs=2)
                        nc.vector.tensor_tensor(out=cexc[:], in0=cinc[:], in1=ctot[:],
                                                op=mybir.AluOpType.subtract)
                        nc.tensor.matmul(out=rps[:], lhsT=ones_row1[:], rhs=cexc[:],
                                         start=False, stop=True)
                        rank = rt.tile([P, 2 * NCOL], f32, tag="rank", bufs=2)
                        nc.vector.tensor_copy(rank[:], rps[:])
                        # sel = kept ? e*EPAD + rank : DUMMY
                        kept = rt.tile([P, 2 * NCOL], f32, tag="kept", bufs=2)
                        nc.vector.tensor_scalar(out=kept[:], in0=rank[:],
                                                scalar1=float(EPAD), scalar2=None,
                                                op0=mybir.AluOpType.is_lt)
                        sel = rt.tile([P, 2 * NCOL], f32, tag="sel", bufs=2)
                        nc.vector.tensor_scalar(out=sel[:], in0=rank[:],
                                                scalar1=float(e * EPAD), scalar2=None,
                                                op0=mybir.AluOpType.add)
                        nc.vector.tensor_tensor(out=sel[:], in0=sel[:], in1=kept[:],
                                                op=mybir.AluOpType.mult)
                        nc.vector.tensor_scalar(out=kept[:], in0=kept[:],
                                                scalar1=-float(DUMMY),
                                                scalar2=float(DUMMY),
                                                op0=mybir.AluOpType.mult,
                                                op1=mybir.AluOpType.add)
                        nc.vector.tensor_tensor(out=sel[:], in0=sel[:], in1=kept[:],
                                                op=mybir.AluOpType.add)
                        msel = rt.tile([P, 2 * NCOL], f32, tag="msel", bufs=2)
                        nc.vector.tensor_tensor(out=msel[:], in0=mask[:], in1=sel[:],
                                                op=mybir.AluOpType.mult)
                        nc.vector.tensor_tensor(out=acc0[:], in0=acc0[:],
                                                in1=msel[:, 0:NCOL],
                                                op=mybir.AluOpType.add)
                        nc.vector.tensor_tensor(out=acc1[:], in0=acc1[:],
                                                in1=msel[:, NCOL:2 * NCOL],
                                                op=mybir.AluOpType.add)

                    nc.vector.tensor_copy(gi0[:], acc0[:])
                    nc.vector.tensor_copy(gi1[:], acc1[:])

                    # ----- scatter (tok, gate) into slotmeta by slot -----
                    zslot = rt.tile([P, 2 * NSLOT_T], f32)
                    nc.gpsimd.memset(zslot[:], 0.0)
                    nc.gpsimd.dma_start(
                        out=slotmeta.ap().rearrange("(p s) w -> p (s w)", p=P),
                        in_=zslot[:])
                    for kk, (acc, G) in enumerate(((acc0, G1), (acc1, G2))):
                        ri = rt.tile([P, NCOL], i32, tag="ri", bufs=2)
                        nc.vector.tensor_copy(ri[:], acc[:])
                        rand_ = rt.tile([P, NCOL], i32, tag="rand", bufs=2)
                        nc.vector.tensor_scalar(out=rand_[:], in0=ri[:], scalar1=127,
                                                scalar2=None,
                                                op0=mybir.AluOpType.bitwise_and)
                        nc.vector.tensor_scalar(out=rand_[:], in0=rand_[:],
                                                scalar1=NSLOT_T, scalar2=None,
                                                op0=mybir.AluOpType.mult)
                        rshr = rt.tile([P, NCOL], i32, tag="rshr", bufs=2)
                        nc.vector.tensor_scalar(out=rshr[:], in0=ri[:], scalar1=7,
                                                scalar2=None,
                                                op0=mybir.AluOpType.logical_shift_right)
                        dst = rt.tile([P, NCOL], i32, tag="dst", bufs=2)
                        nc.vector.tensor_tensor(out=dst[:], in0=rand_[:], in1=rshr[:],
                                                op=mybir.AluOpType.add)
                        # overflow (acc==DUMMY) -> far out of bounds, dropped
                        oob = rt.tile([P, NCOL], f32, tag="oob", bufs=2)
                        nc.vector.tensor_scalar(out=oob[:], in0=acc[:],
                                                scalar1=float(DUMMY), scalar2=None,
                                                op0=mybir.AluOpType.is_ge)
                        oobi = rt.tile([P, NCOL], i32, tag="oobi", bufs=2)
                        nc.vector.tensor_scalar(out=oob[:], in0=oob[:],
                                                scalar1=1.0e6, scalar2=None,
                                                op0=mybir.AluOpType.mult)
                        nc.vector.tensor_copy(oobi[:], oob[:])
                        nc.vector.tensor_tensor(out=dst[:], in0=dst[:], in1=oobi[:],
                                                op=mybir.AluOpType.add)
                        pay = rt.tile([P, 2 * NCOL], f32, tag="pay", bufs=2)
                        nc.vector.tensor_copy(pay[:, 0:2 * NCOL:2], tokf[:])
                        nc.vector.tensor_copy(pay[:, 1:2 * NCOL:2], G[:])
                        for c in range(NCOL):
                            nc.gpsimd.indirect_dma_start(
                                out=slotmeta.ap(),
                                out_offset=bass.IndirectOffsetOnAxis(
                                    ap=dst[:, c:c + 1], axis=0),
                                in_=pay[:, 2 * c:2 * c + 2],
                                in_offset=None,
                                bounds_check=NSLOT - 1,
                                oob_is_err=False,
                            )

                    # ----- read back compacted slot meta -----
                    smf = rt.tile([P, NSLOT_T, 2], f32)
                    nc.sync.dma_start(out=smf[:], in_=slotmeta.ap().rearrange(
                        "(p s) w -> p s w", p=P))
                    nc.vector.tensor_copy(slot_w[:], smf[:, :, 1])
                    nc.vector.tensor_copy(slot_tok[:], smf[:, :, 0])

                # =========== expert FFN (all experts, own tokens) ===========
                # zero the dummy eo rows once
                with tc.tile_pool(name="ffn", bufs=1) as ffn, \
                     tc.tile_pool(name="xgp", bufs=3) as xgp, \
                     tc.tile_pool(name="xtp", bufs=2) as xtp, \
                     tc.tile_pool(name="owp", bufs=3) as owp, \
                     tc.tile_pool(name="ffp", bufs=2, space="PSUM") as ffp:
                    zeo = ffn.tile([P, D], bf16)
                    nc.vector.memset(zeo[:], 0.0)
                    nc.scalar.dma_start(out=eo_d.ap()[NSLOT:NSLOT + P, :], in_=zeo[:])

                    for e in range(E):
                        w1t = w1p.tile([P, DC, FC, P], bf16, tag="w1")
                        nc.sync.dma_start(out=w1t[:], in_=w1b.ap()[e].rearrange(
                            "p (a b q) -> p a b q", a=DC, b=FC))
                        w2t = w2p.tile([P, FC, DC, P], bf16, tag="w2")
                        nc.sync.dma_start(out=w2t[:], in_=w2b.ap()[e].rearrange(
                            "p (a b q) -> p a b q", a=FC, b=DC))

                        # gather + transpose own-token rows for this expert
                        xgT = xtp.tile([P, DC, EPAD], bf16, tag="xgT")
                        for t in range(ETIL):
                            xg = xgp.tile([P, D], f32, tag="xg")
                            nc.gpsimd.indirect_dma_start(
                                out=xg[:], out_offset=None,
                                in_=x_s.ap(),
                                in_offset=bass.IndirectOffsetOnAxis(
                                    ap=slot_tok[:, e * ETIL + t:e * ETIL + t + 1],
                                    axis=0),
                                bounds_check=TPC - 1,
                                oob_is_err=False,
                            )
                            for dci in range(DC):
                                tp = ffp.tile([P, P], f32, space="PSUM", tag="tp")
                                nc.tensor.transpose(out=tp[:],
                                                    in_=xg[:, dci * P:(dci + 1) * P],
                                                    identity=ident[:])
                                nc.vector.tensor_copy(
                                    xgT[:, dci, t * P:(t + 1) * P], tp[:])

                        # mm1 + gelu -> hT
                        hT = ffn.tile([P, FC, EPAD], bf16, tag="hT", bufs=2)
                        for fci in range(FC):
                            hp = ffp.tile([P, EPAD], f32, space="PSUM", tag="hp")
                            for dci in range(DC):
                                nc.tensor.matmul(out=hp[:],
                                                 lhsT=w1t[:, dci, fci, :],
                                                 rhs=xgT[:, dci, :],
                                                 start=(dci == 0), stop=(dci == DC - 1))
                            nc.scalar.activation(out=hT[:, fci, :], in_=hp[:],
                                                 func=mybir.ActivationFunctionType.Gelu,
                                                 bias=b1_sb[:, e * FC + fci:e * FC + fci + 1],
                                                 scale=1.0)
                        # mm2 (+bias) -> oT
                        oT = ffn.tile([P, DC, EPAD], bf16, tag="oT", bufs=2)
                        for dci in range(DC):
                            op = ffp.tile([P, EPAD], f32, space="PSUM", tag="op")
                            for fci in range(FC):
                                nc.tensor.matmul(out=op[:],
                                                 lhsT=w2t[:, fci, dci, :],
                                                 rhs=hT[:, fci, :],
                                                 start=(fci == 0), stop=(fci == FC - 1))
                            nc.vector.tensor_scalar(
                                out=oT[:, dci, :], in0=op[:],
                                scalar1=b2_sb[:, e * DC + dci:e * DC + dci + 1],
                                scalar2=None, op0=mybir.AluOpType.add)
                        # transpose back, gate, store
                        for cc in range(ETIL):
                            ow = owp.tile([P, D], bf16, tag="ow")
                            for dci in range(DC):
                                tp2 = ffp.tile([P, P], bf16, space="PSUM", tag="tp2")
                                nc.tensor.transpose(out=tp2[:],
                                                    in_=oT[:, dci, cc * P:(cc + 1) * P],
                                                    identity=identb[:])
                                nc.scalar.activation(
                                    out=ow[:, dci * P:(dci + 1) * P], in_=tp2[:],
                                    func=mybir.ActivationFunctionType.Copy,
                                    scale=slot_w[:, e * ETIL + cc:e * ETIL + cc + 1])
                            row0 = e * EPAD + cc * P
                            nc.scalar.dma_start(out=eo_d.ap()[row0:row0 + P, :],
                                                in_=ow[:])

            # =========== local combine ===========
            with tc.tile_pool(name="cmb", bufs=3) as cmb:
                for c in range(NCOL):
                    cb0 = cmb.tile([P, D], bf16, tag="cb0")
                    cb1 = cmb.tile([P, D], bf16, tag="cb1")
                    nc.gpsimd.indirect_dma_start(
                        out=cb0[:], out_offset=None, in_=eo_d.ap(),
                        in_offset=bass.IndirectOffsetOnAxis(ap=gi0[:, c:c + 1], axis=0),
                    )
                    nc.gpsimd.indirect_dma_start(
                        out=cb1[:], out_offset=None, in_=eo_d.ap(),
                        in_offset=bass.IndirectOffsetOnAxis(ap=gi1[:, c:c + 1], axis=0),
                    )
                    osb = cmb.tile([P, D], f32, tag="osb")
                    nc.vector.tensor_tensor(out=osb[:], in0=cb0[:], in1=cb1[:],
                                            op=mybir.AluOpType.add)
                    nc.sync.dma_start(out=out_shard.ap()[c * P:(c + 1) * P, :],
                                      in_=osb[:])

    nc.finalize()
    return nc


_NC_CACHE = None
TRACE = False
LAST_EXEC_NS = None
LAST_TRACE_DIR = None


def kernel(x, router_w, router_b, w1, b1, w2, b2):
    global _NC_CACHE, LAST_EXEC_NS, LAST_TRACE_DIR
    import ml_dtypes
    bf = ml_dtypes.bfloat16

    x = np.ascontiguousarray(np.asarray(x, np.float32))
    router_w = np.ascontiguousarray(np.asarray(router_w, np.float32))
    router_b = np.asarray(router_b, np.float32)
    w1 = np.asarray(w1, np.float32)
    b1 = np.asarray(b1, np.float32)
    w2 = np.asarray(w2, np.float32)
    b2 = np.asarray(b2, np.float32)

    xf = x.reshape(T, D)
    rb_b = np.ascontiguousarray(np.tile(router_b[None, :], (P, 1)))
    rw_t = np.ascontiguousarray(
        router_w.reshape(DC, P, E).transpose(1, 0, 2).reshape(P, DC * E))
    w1b = np.ascontiguousarray(
        w1.reshape(E, DC, P, FC, P).transpose(0, 2, 1, 3, 4)
        .reshape(E, P, DC * FC * P).astype(bf))
    w2b = np.ascontiguousarray(
        w2.reshape(E, FC, P, DC, P).transpose(0, 2, 1, 3, 4)
        .reshape(E, P, FC * DC * P).astype(bf))
    b1t = np.ascontiguousarray(
        b1.reshape(E, FC, P).transpose(2, 0, 1).reshape(P, E * FC))
    b2t = np.ascontiguousarray(
        b2.reshape(E, DC, P).transpose(2, 0, 1).reshape(P, E * DC))

    in_maps = []
    for c in range(N_CORES):
        xs = np.ascontiguousarray(xf[c * TPC:(c + 1) * TPC])
        xT = np.ascontiguousarray(
            xs.T.reshape(DC, P, TPC).transpose(1, 0, 2).reshape(P, DC * TPC))
        in_maps.append({
            "x_s": xs,
            "xT_s": xT,
            "rw_t": rw_t,
            "rb_b": rb_b,
            "w1b": w1b,
            "w2b": w2b,
            "b1t": b1t,
            "b2t": b2t,
        })

    if _NC_CACHE is None:
        _NC_CACHE = build_kernel()
    import tempfile
    td = tempfile.mkdtemp(prefix="moe_trace_") if TRACE else None
    res = run_bass_kernel_spmd(_NC_CACHE, in_maps, list(range(N_CORES)),
                               trace=TRACE, tmpdir=td)
    LAST_EXEC_NS = getattr(res, "exec_time_ns", None)
    LAST_TRACE_DIR = td
    out = np.concatenate([res.results[c]["out_shard"] for c in range(N_CORES)], axis=0)
    return out.reshape(B, S, D)


# revision 10
# speedup vs baseline: 1.8991x; 1.0013x over previous
"""MoE feed-forward (top-2 routing, E=8 experts) on 8 trn2 NeuronCores.

Strategy: token parallelism with zero collectives.
  - Core c owns tokens [1024*c, 1024*(c+1)) and computes their complete MoE
    output locally: router -> per-expert compaction -> all 8 experts' FFN on
    its own tokens -> scatter-add combine.  No cross-core communication.
  - Capacity drops never occur for this problem size (max expert load ~2151
    vs CAP 2560, 12 sigma of margin), so routing is purely local; a safety
    clamp drops any hypothetical overflow via DMA bounds checks.
  - Per (core, expert) token counts are ~256 +- 15; each expert group is
    padded to EPAD=384 slots (3 tiles of 128).  Padded slots compute token 0
    with gate 0 and contribute nothing.
  - FFN in bf16 (weights streamed from DRAM as pre-laid-out bf16 on the sync
    DMA queue, double-buffered; x gathered fp32 then cast during the
    transpose drain), fp32 PSUM accumulate.  Router logits in full fp32 to
    reproduce the reference's top-2 selection exactly.
  - Combine: gated expert-output rows are indirect-DMA scatter-ADDed
    (CCE add) straight into the zeroed fp32 output shard during the FFN,
    so there is no combine tail.

Token layout on-chip: [128 partitions, 8 columns], local token t = 128*c + p.
Slot r of expert e maps to slotmeta row (r'%128)*24 + r'//128, r' = e*384+r.
"""
import numpy as np

import concourse.tile as tile
from concourse import bass, bacc, mybir
from concourse.bass_utils import run_bass_kernel_spmd
from concourse.masks import make_identity, make_upper_triangular

N_CORES = 8
P = 128
E = 8
K = 2
D = 1024
F = 2048
B, S = 4, 2048
T = B * S                  # 8192 tokens
TPC = T // N_CORES         # 1024 tokens per core
NCOL = TPC // P            # 8 token columns
DC = D // P                # 8 d-chunks
FC = F // P                # 16 f-chunks
EPAD = 384                 # padded slots per expert (max observed count ~287)
ETIL = EPAD // P           # 3 slot tiles per expert
NSLOT = E * EPAD           # 3072 total slots
NSLOT_T = NSLOT // P       # 24 slot tiles
f32 = mybir.dt.float32
bf16 = mybir.dt.bfloat16
i32 = mybir.dt.int32
ADD = mybir.AluOpType.add
SUB = mybir.AluOpType.subtract
MUL = mybir.AluOpType.mult


def build_kernel():
    nc = bacc.Bacc(num_devices=N_CORES)

    # ---------------- parameters (host pre-laid-out) ----------------
    x_s = nc.declare_dram_parameter("x_s", [TPC, D], f32, isOutput=False)
    xT_s = nc.declare_dram_parameter("xT_s", [P, DC * TPC], f32, isOutput=False)
    rw_t = nc.declare_dram_parameter("rw_t", [P, DC * E], f32, isOutput=False)
    rb_r = nc.declare_dram_parameter("rb_r", [P, NCOL * E], f32, isOutput=False)
    w1b = nc.declare_dram_parameter("w1b", [E, P, DC * FC * P], bf16, isOutput=False)
    w2b = nc.declare_dram_parameter("w2b", [E, P, FC * DC * P], bf16, isOutput=False)
    b1t = nc.declare_dram_parameter("b1t", [P, E * FC], f32, isOutput=False)
    b2t = nc.declare_dram_parameter("b2t", [P, E * DC], f32, isOutput=False)
    out_shard = nc.declare_dram_parameter("out_shard", [TPC, D], f32, isOutput=True)

    # ---------------- internal DRAM ----------------
    slotmeta = nc.dram_tensor("slotmeta", [NSLOT, 2], f32)   # (tok, gate) by slot

    with tile.TileContext(nc) as tc:
        with tc.tile_pool(name="const", bufs=1) as cpool:
            rw_sb = cpool.tile([P, DC, E], f32)
            nc.sync.dma_start(out=rw_sb[:], in_=rw_t.ap().rearrange(
                "p (c e) -> p c e", c=DC))
            rb_sb = cpool.tile([P, NCOL * E], f32)
            nc.sync.dma_start(out=rb_sb[:], in_=rb_r.ap())
            b1_sb = cpool.tile([P, E * FC], f32)
            nc.sync.dma_start(out=b1_sb[:], in_=b1t.ap())
            b2_sb = cpool.tile([P, E * DC], f32)
            nc.sync.dma_start(out=b2_sb[:], in_=b2t.ap())

            ident = cpool.tile([P, P], f32)
            make_identity(nc, ident[:])
            identb = cpool.tile([P, P], bf16)
            nc.vector.tensor_copy(identb[:], ident[:])
            tri = cpool.tile([P, P], f32)
            make_upper_triangular(nc, tri[:], val=1.0, diag=False)  # tri[p,i]=1 iff p<i
            ones_col = cpool.tile([P, 1], f32)
            nc.gpsimd.memset(ones_col[:], 1.0)
            ones_row1 = cpool.tile([1, P], f32)
            nc.gpsimd.memset(ones_row1[:], 1.0)
            toki = cpool.tile([P, NCOL], i32)
            nc.gpsimd.iota(toki[:], pattern=[[P, NCOL]], base=0, channel_multiplier=1)
            tokf = cpool.tile([P, NCOL], f32)
            nc.vector.tensor_copy(tokf[:], toki[:])
            eidxi = cpool.tile([P, E * 2 * NCOL], i32)
            nc.gpsimd.iota(eidxi[:], pattern=[[1, E], [0, 2 * NCOL]], base=0,
                           channel_multiplier=0)
            eidxf = cpool.tile([P, E * 2 * NCOL], f32)
            nc.vector.tensor_copy(eidxf[:], eidxi[:])
            ebase = cpool.tile([P, E * 2 * NCOL], f32)
            nc.vector.tensor_scalar(out=ebase[:], in0=eidxf[:], scalar1=float(EPAD),
                                    scalar2=None, op0=MUL)
            # persistent routing results
            slot_tok = cpool.tile([P, NSLOT_T], i32)
            slot_w = cpool.tile([P, NSLOT_T], f32)

            with tc.tile_pool(name="w1p", bufs=2) as w1p, \
                 tc.tile_pool(name="w2p", bufs=2) as w2p:

                # =========== routing (local tokens only) ===========
                with tc.tile_pool(name="rt", bufs=1) as rt, \
                     tc.tile_pool(name="rtp", bufs=2, space="PSUM") as rtp:
                    # sync queue: xT first (router critical path)
                    xT_r = rt.tile([P, DC, TPC], f32)
                    nc.sync.dma_start(out=xT_r[:], in_=xT_s.ap().rearrange(
                        "p (c t) -> p c t", c=DC))
                    # zero the output shard early (scalar queue, idle at start)
                    zot = rt.tile([P, D], f32)
                    nc.vector.memset(zot[:], 0.0)
                    for c in range(NCOL):
                        nc.scalar.dma_start(out=out_shard.ap()[c * P:(c + 1) * P, :],
                                            in_=zot[:])
                    lg_sb = rt.tile([E, TPC], f32)
                    for h in range(2):
                        lgp = rtp.tile([E, TPC // 2], f32, space="PSUM", tag="lgp")
                        for dci in range(DC):
                            nc.tensor.matmul(
                                out=lgp[:], lhsT=rw_sb[:, dci, :],
                                rhs=xT_r[:, dci, h * (TPC // 2):(h + 1) * (TPC // 2)],
                                start=(dci == 0), stop=(dci == DC - 1))
                        nc.vector.tensor_copy(lg_sb[:, h * (TPC // 2):(h + 1) * (TPC // 2)],
                                              lgp[:])
                    # batched top-2: transpose all columns, then wide DVE ops
                    lsb = rt.tile([P, NCOL * E], f32)
                    for c in range(NCOL):
                        ltp = rtp.tile([P, E], f32, space="PSUM", tag="ltp")
                        nc.tensor.transpose(out=ltp[:],
                                            in_=lg_sb[:, c * P:(c + 1) * P],
                                            identity=ident[0:E, 0:E])
                        nc.vector.tensor_copy(lsb[:, c * E:(c + 1) * E], ltp[:])
                    nc.vector.tensor_tensor(out=lsb[:], in0=lsb[:], in1=rb_sb[:],
                                            op=ADD)
                    mxa = rt.tile([P, NCOL * 8], f32)
                    mia = rt.tile([P, NCOL * 8], mybir.dt.uint32)
                    for c in range(NCOL):
                        nc.vector.max_with_indices(mxa[:, c * 8:(c + 1) * 8],
                                                   mia[:, c * 8:(c + 1) * 8],
                                                   lsb[:, c * E:(c + 1) * E])
                    E1f = rt.tile([P, NCOL], f32)
                    E2f = rt.tile([P, NCOL], f32)
                    nc.vector.tensor_copy(E1f[:], mia[:, 0:NCOL * 8:8])
                    nc.vector.tensor_copy(E2f[:], mia[:, 1:NCOL * 8:8])
                    G1 = rt.tile([P, NCOL], f32)
                    G2 = rt.tile([P, NCOL], f32)
                    diff = rt.tile([P, NCOL], f32)
                    nc.vector.tensor_tensor(out=diff[:], in0=mxa[:, 0:NCOL * 8:8],
                                            in1=mxa[:, 1:NCOL * 8:8], op=SUB)
                    nc.scalar.activation(out=G1[:], in_=diff[:],
                                         func=mybir.ActivationFunctionType.Sigmoid)
                    nc.vector.tensor_scalar(out=G2[:], in0=G1[:],
                                            scalar1=-1.0, scalar2=1.0,
                                            op0=MUL, op1=ADD)

                    # ----- batched per-expert ranks -----
                    NW = E * 2 * NCOL      # 128 work columns: e-major, j=(k,c)
                    erep = rt.tile([P, NW], f32)
                    for e in range(E):
                        nc.vector.tensor_copy(erep[:, e * 2 * NCOL:e * 2 * NCOL + NCOL],
                                              E1f[:])
                        nc.vector.tensor_copy(
                            erep[:, e * 2 * NCOL + NCOL:(e + 1) * 2 * NCOL], E2f[:])
                    mask = rt.tile([P, NW], f32)
                    nc.vector.tensor_tensor(out=mask[:], in0=erep[:], in1=eidxf[:],
                                            op=mybir.AluOpType.is_equal)
                    rps = rtp.tile([P, NW], f32, space="PSUM", tag="rps")
                    nc.tensor.matmul(out=rps[:], lhsT=tri[:], rhs=mask[:],
                                     start=True, stop=False)
                    cps = rtp.tile([1, NW], f32, space="PSUM", tag="cps")
                    nc.tensor.matmul(out=cps[:], lhsT=ones_col[:], rhs=mask[:],
                                     start=True, stop=True)
                    ctot = rt.tile([1, NW], f32)
                    nc.vector.tensor_copy(ctot[:], cps[:])
                    cinc = rt.tile([1, NW], f32)
                    for e in range(E):
                        sl = slice(e * 2 * NCOL, (e + 1) * 2 * NCOL)
                        nc.vector.tensor_tensor_scan(
                            out=cinc[:, sl], data0=ctot[:, sl], data1=ctot[:, sl],
                            initial=0.0, op0=ADD, op1=mybir.AluOpType.bypass)
                    cexc = rt.tile([1, NW], f32)
                    nc.vector.tensor_tensor(out=cexc[:], in0=cinc[:], in1=ctot[:],
                                            op=SUB)
                    nc.tensor.matmul(out=rps[:], lhsT=ones_row1[:], rhs=cexc[:],
                                     start=False, stop=True)
                    rank = rt.tile([P, NW], f32)
                    nc.vector.tensor_copy(rank[:], rps[:])
                    kept = rt.tile([P, NW], f32)
                    nc.vector.tensor_scalar(out=kept[:], in0=rank[:],
                                            scalar1=float(EPAD), scalar2=None,
                                            op0=mybir.AluOpType.is_lt)
                    valid = rt.tile([P, NW], f32)
                    nc.vector.tensor_tensor(out=valid[:], in0=mask[:], in1=kept[:],
                                            op=MUL)
                    sel = rt.tile([P, NW], f32)
                    nc.vector.tensor_tensor(out=sel[:], in0=rank[:], in1=ebase[:],
                                            op=ADD)
                    nc.vector.tensor_tensor(out=sel[:], in0=sel[:], in1=valid[:],
                                            op=MUL)
                    # acc[k] = slot index per assignment; vsum = 1 unless overflow
                    acc = rt.tile([P, 2 * NCOL], f32)
                    vsum = rt.tile([P, 2 * NCOL], f32)
                    nc.vector.memset(acc[:], 0.0)
                    nc.vector.memset(vsum[:], 0.0)
                    for e in range(E):
                        sl = slice(e * 2 * NCOL, (e + 1) * 2 * NCOL)
                        nc.vector.tensor_tensor(out=acc[:], in0=acc[:],
                                                in1=sel[:, sl], op=ADD)
                        nc.vector.tensor_tensor(out=vsum[:], in0=vsum[:],
                                                in1=valid[:, sl], op=ADD)
                    # overflow -> push dst far out of bounds (dropped by DMA)
                    nc.vector.tensor_scalar(out=vsum[:], in0=vsum[:],
                                            scalar1=-1.0e6, scalar2=1.0e6,
                                            op0=MUL, op1=ADD)
                    nc.vector.tensor_tensor(out=acc[:], in0=acc[:], in1=vsum[:],
                                            op=ADD)
                    # dst row = (slot%128)*24 + slot//128
                    ri = rt.tile([P, 2 * NCOL], i32)
                    nc.vector.tensor_copy(ri[:], acc[:])
                    rand_ = rt.tile([P, 2 * NCOL], i32)
                    nc.vector.tensor_scalar(out=rand_[:], in0=ri[:], scalar1=127,
                                            scalar2=None,
                                            op0=mybir.AluOpType.bitwise_and)
                    nc.vector.tensor_scalar(out=rand_[:], in0=rand_[:],
                                            scalar1=NSLOT_T, scalar2=None, op0=MUL)
                    rshr = rt.tile([P, 2 * NCOL], i32)
                    nc.vector.tensor_scalar(out=rshr[:], in0=ri[:], scalar1=7,
                                            scalar2=None,
                                            op0=mybir.AluOpType.logical_shift_right)
                    dst = rt.tile([P, 2 * NCOL], i32)
                    nc.vector.tensor_tensor(out=dst[:], in0=rand_[:], in1=rshr[:],
                                            op=ADD)
                    pay = rt.tile([P, 4 * NCOL], f32)
                    nc.vector.tensor_copy(pay[:, 0:2 * NCOL:2], tokf[:])
                    nc.vector.tensor_copy(pay[:, 1:2 * NCOL:2], G1[:])
                    nc.vector.tensor_copy(pay[:, 2 * NCOL:4 * NCOL:2], tokf[:])
                    nc.vector.tensor_copy(pay[:, 2 * NCOL + 1:4 * NCOL:2], G2[:])

                    # zero slotmeta, scatter, read back (all on gpsimd queue)
                    # padded slots: token 2048 (out of bounds -> DMA-dropped), gate 0
                    zslot = rt.tile([P, 2 * NSLOT_T], f32)
                    nc.gpsimd.memset(zslot[:, 0:2 * NSLOT_T:2], 2.0 * TPC)
                    nc.gpsimd.memset(zslot[:, 1:2 * NSLOT_T:2], 0.0)
                    nc.gpsimd.dma_start(
                        out=slotmeta.ap().rearrange("(p s) w -> p (s w)", p=P),
                        in_=zslot[:])
                    for j in range(2 * NCOL):
                        nc.gpsimd.indirect_dma_start(
                            out=slotmeta.ap(),
                            out_offset=bass.IndirectOffsetOnAxis(
                                ap=dst[:, j:j + 1], axis=0),
                            in_=pay[:, 2 * j:2 * j + 2],
                            in_offset=None,
                            bounds_check=NSLOT - 1,
                            oob_is_err=False,
                        )
                    smf = rt.tile([P, NSLOT_T, 2], f32)
                    nc.gpsimd.dma_start(out=smf[:], in_=slotmeta.ap().rearrange(
                        "(p s) w -> p s w", p=P))
                    nc.vector.tensor_copy(slot_w[:], smf[:, :, 1])
                    nc.vector.tensor_copy(slot_tok[:], smf[:, :, 0])

                # =========== expert FFN + scatter-add combine ===========
                with tc.tile_pool(name="ffn", bufs=1) as ffn, \
                     tc.tile_pool(name="xgp", bufs=4) as xgp, \
                     tc.tile_pool(name="xtp", bufs=2) as xtp, \
                     tc.tile_pool(name="owp", bufs=2) as owp, \
                     tc.tile_pool(name="ffp", bufs=2, space="PSUM") as ffp:

                    def issue_gathers(e):
                        tiles = []
                        for t in range(ETIL):
                            xg = xgp.tile([P, D], f32, tag="xg")
                            nc.gpsimd.indirect_dma_start(
                                out=xg[:], out_offset=None,
                                in_=x_s.ap(),
                                in_offset=bass.IndirectOffsetOnAxis(
                                    ap=slot_tok[:, e * ETIL + t:e * ETIL + t + 1],
                                    axis=0),
                                bounds_check=TPC - 1,
                                oob_is_err=False,
                            )
                            tiles.append(xg)
                        return tiles

                    pend = issue_gathers(0)
                    for e in range(E):
                        w1t = w1p.tile([P, DC, FC, P], bf16, tag="w1")
                        nc.sync.dma_start(out=w1t[:], in_=w1b.ap()[e].rearrange(
                            "p (a b q) -> p a b q", a=DC, b=FC))
                        w2t = w2p.tile([P, FC, DC, P], bf16, tag="w2")
                        nc.sync.dma_start(out=w2t[:], in_=w2b.ap()[e].rearrange(
                            "p (a b q) -> p a b q", a=FC, b=DC))

                        cur = pend
                        if e + 1 < E:
                            pend = issue_gathers(e + 1)

                        # transpose + cast gathered rows -> xgT [d, slot]
                        xgT = xtp.tile([P, DC, EPAD], bf16, tag="xgT")
                        for t, xg in enumerate(cur):
                            for dci in range(DC):
                                tp = ffp.tile([P, P], f32, space="PSUM", tag="tp")
                                nc.tensor.transpose(out=tp[:],
                                                    in_=xg[:, dci * P:(dci + 1) * P],
                                                    identity=ident[:])
                                nc.vector.tensor_copy(
                                    xgT[:, dci, t * P:(t + 1) * P], tp[:])

                        # mm1 + gelu -> hT
                        hT = ffn.tile([P, FC, EPAD], bf16, tag="hT", bufs=1)
                        for fci in range(FC):
                            hp = ffp.tile([P, EPAD], f32, space="PSUM", tag="hp")
                            for dci in range(DC):
                                nc.tensor.matmul(out=hp[:],
                                                 lhsT=w1t[:, dci, fci, :],
                                                 rhs=xgT[:, dci, :],
                                                 start=(dci == 0), stop=(dci == DC - 1))
                            nc.scalar.activation(out=hT[:, fci, :], in_=hp[:],
                                                 func=mybir.ActivationFunctionType.Gelu,
                                                 bias=b1_sb[:, e * FC + fci:e * FC + fci + 1],
                                                 scale=1.0)
                        # mm2 (+bias) -> oT
                        oT = ffn.tile([P, DC, EPAD], bf16, tag="oT", bufs=2)
                        for dci in range(DC):
                            op = ffp.tile([P, EPAD], f32, space="PSUM", tag="op")
                            for fci in range(FC):
                                nc.tensor.matmul(out=op[:],
                                                 lhsT=w2t[:, fci, dci, :],
                                                 rhs=hT[:, fci, :],
                                                 start=(fci == 0), stop=(fci == FC - 1))
                            nc.vector.tensor_scalar(
                                out=oT[:, dci, :], in0=op[:],
                                scalar1=b2_sb[:, e * DC + dci:e * DC + dci + 1],
                                scalar2=None, op0=ADD)
                        # transpose back, gate, scatter-add into output
                        for cc in range(ETIL):
                            ow = owp.tile([P, D], f32, tag="ow")
                            for dci in range(DC):
                                tp2 = ffp.tile([P, P], bf16, space="PSUM", tag="tp2")
                                nc.tensor.transpose(out=tp2[:],
                                                    in_=oT[:, dci, cc * P:(cc + 1) * P],
                                                    identity=identb[:])
                                nc.scalar.activation(
                                    out=ow[:, dci * P:(dci + 1) * P], in_=tp2[:],
                                    func=mybir.ActivationFunctionType.Copy,
                                    scale=slot_w[:, e * ETIL + cc:e * ETIL + cc + 1])
                            nc.gpsimd.indirect_dma_start(
                                out=out_shard.ap(),
                                out_offset=bass.IndirectOffsetOnAxis(
                                    ap=slot_tok[:, e * ETIL + cc:e * ETIL + cc + 1],
                                    axis=0),
                                in_=ow[:],
                                in_offset=None,
                                bounds_check=TPC - 1,
                                oob_is_err=False,
                                compute_op=ADD,
                            )

    nc.finalize()
    return nc


_NC_CACHE = None
TRACE = False
LAST_EXEC_NS = None
LAST_TRACE_DIR = None


def kernel(x, router_w, router_b, w1, b1, w2, b2):
    global _NC_CACHE, LAST_EXEC_NS, LAST_TRACE_DIR
    import ml_dtypes
    bf = ml_dtypes.bfloat16

    x = np.ascontiguousarray(np.asarray(x, np.float32))
    router_w = np.ascontiguousarray(np.asarray(router_w, np.float32))
    router_b = np.asarray(router_b, np.float32)
    w1 = np.asarray(w1, np.float32)
    b1 = np.asarray(b1, np.float32)
    w2 = np.asarray(w2, np.float32)
    b2 = np.asarray(b2, np.float32)

    xf = x.reshape(T, D)
    rb_r = np.ascontiguousarray(np.tile(router_b, (P, NCOL)))
    rw_t = np.ascontiguousarray(
        router_w.reshape(DC, P, E).transpose(1, 0, 2).reshape(P, DC * E))
    w1b = np.ascontiguousarray(
        w1.reshape(E, DC, P, FC, P).transpose(0, 2, 1, 3, 4)
        .reshape(E, P, DC * FC * P).astype(bf))
    w2b = np.ascontiguousarray(
        w2.reshape(E, FC, P, DC, P).transpose(0, 2, 1, 3, 4)
        .reshape(E, P, FC * DC * P).astype(bf))
    b1t = np.ascontiguousarray(
        b1.reshape(E, FC, P).transpose(2, 0, 1).reshape(P, E * FC))
    b2t = np.ascontiguousarray(
        b2.reshape(E, DC, P).transpose(2, 0, 1).reshape(P, E * DC))

    in_maps = []
    for c in range(N_CORES):
        xs = np.ascontiguousarray(xf[c * TPC:(c + 1) * TPC])
        xT = np.ascontiguousarray(
            xs.T.reshape(DC, P, TPC).transpose(1, 0, 2).reshape(P, DC * TPC))
        in_maps.append({
            "x_s": xs,
            "xT_s": xT,
            "rw_t": rw_t,
            "rb_r": rb_r,
            "w1b": w1b,
            "w2b": w2b,
            "b1t": b1t,
            "b2t": b2t,
        })

    if _NC_CACHE is None:
        _NC_CACHE = build_kernel()
    import tempfile
    td = tempfile.mkdtemp(prefix="moe_trace_") if TRACE else None
    res = run_bass_kernel_spmd(_NC_CACHE, in_maps, list(range(N_CORES)),
                               trace=TRACE, tmpdir=td)
    LAST_EXEC_NS = getattr(res, "exec_time_ns", None)
    LAST_TRACE_DIR = td
    out = np.concatenate([res.results[c]["out_shard"] for c in range(N_CORES)], axis=0)
    return out.reshape(B, S, D)
